# revision 2
# baseline (speedup 1.0000x reference)
"""CRF Viterbi decode (tf.contrib.crf.crf_decode + one_hot) on 8 TRN2 cores.

Data-parallel over batch: each of the 8 NeuronCores processes 128 of the
1024 sequences (batch rows on SBUF partitions). Per core, the whole DP is
SBUF-resident:

  forward (t = 1..511), all on the Vector engine:
    x[b,cc,cp] = A_T[cc,cp] + s[b,cp]            tensor_tensor add (s broadcast)
    raw[b,cc]  = max_cp x                        tensor_reduce axis=X
    eq         = (x == raw)                      tensor_tensor is_equal
    v          = eq * (48-cp)                    tensor_tensor mult
    bpv[b,cc]  = max_cp v                        tensor_reduce  (=48-argmax,
                                                 first-index tie-break like jnp)
    bp[t-1]    = u8(bpv)                         tensor_copy (encoded 48-argmax)
    bp[t-1]    = 48-cp where t >= len[b]         copy_predicated (identity fix)
    s          = raw + pot[:,t] where t < len    tensor_tensor + copy_predicated

  last_tag = argmax(s) via Max8/MaxIndex, then a 2-op/step backtrace:
    idx = sum(bp_t * onehot_next)                scalar_tensor_tensor + accum
    onehot = (48-cp == idx)                      tensor_scalar is_equal
  whose one-hot results are written straight into the output chunks.

Scores follow the reference's fp32 arithmetic exactly (same single adds,
exact max, first-index argmax), so the output matches bit-for-bit.
"""
import numpy as np

B, T, C = 1024, 512, 48
NCORES = 8
P = B // NCORES  # 128 batch rows per core
CHUNK = 64

_CACHE = {}


def _build_module():
    from contextlib import ExitStack

    import concourse.bacc as bacc
    import concourse.tile as tile
    from concourse import mybir

    F32 = mybir.dt.float32
    U8 = mybir.dt.uint8
    I32 = mybir.dt.int32
    U32 = mybir.dt.uint32
    ALU = mybir.AluOpType

    nc = bacc.Bacc("TRN2", debug=False, enable_asserts=False,
                   target_bir_lowering=False, num_devices=NCORES)
    pot = nc.dram_tensor("pot", [P, T, C], F32, kind="ExternalInput").ap()
    lens = nc.dram_tensor("lens", [P, 1], F32, kind="ExternalInput").ap()
    at = nc.dram_tensor("at", [P, C, C], F32, kind="ExternalInput").ap()
    outd = nc.dram_tensor("out", [P, T, C], F32, kind="ExternalOutput").ap()

    with tile.TileContext(nc) as tc, ExitStack() as ctx:
        singles = ctx.enter_context(tc.tile_pool(name="singles", bufs=1))
        potp = ctx.enter_context(tc.tile_pool(name="potp", bufs=2))
        xp = ctx.enter_context(tc.tile_pool(name="xp", bufs=2))
        eqp = ctx.enter_context(tc.tile_pool(name="eqp", bufs=2))
        vp = ctx.enter_context(tc.tile_pool(name="vp", bufs=2))
        smal = ctx.enter_context(tc.tile_pool(name="smal", bufs=4))
        outp = ctx.enter_context(tc.tile_pool(name="outp", bufs=2))

        at_sb = singles.tile([P, C, C], F32)
        nc.sync.dma_start(out=at_sb, in_=at)
        lens_sb = singles.tile([P, 1], F32)
        nc.sync.dma_start(out=lens_sb, in_=lens)

        iota_t = singles.tile([P, T], I32)
        nc.gpsimd.iota(iota_t, pattern=[[1, T]], base=0, channel_multiplier=0)
        iota_c = singles.tile([P, C], I32)
        nc.gpsimd.iota(iota_c, pattern=[[1, C]], base=0, channel_multiplier=0)
        invcp = singles.tile([P, C], F32)  # 48 - cp
        nc.vector.tensor_scalar(out=invcp, in0=iota_c, scalar1=-1.0,
                                scalar2=float(C), op0=ALU.mult, op1=ALU.add)
        inv_u8 = singles.tile([P, C], U8)  # identity bp in the 48-cp encoding
        nc.vector.tensor_copy(out=inv_u8, in_=invcp)

        # masks; CopyPredicated needs integer dtype, and only plain
        # tensor_copy may write u8 (other DVE ops w/ u8 out fault the HW)
        mf = singles.tile([P, T], F32)
        nc.vector.tensor_scalar(out=mf, in0=iota_t, scalar1=lens_sb[:, :],
                                scalar2=None, op0=ALU.is_lt)
        m = singles.tile([P, T], U8)
        nc.vector.tensor_copy(out=m, in_=mf)
        minvf = singles.tile([P, T], F32)
        nc.vector.tensor_scalar(out=minvf, in0=mf, scalar1=-1.0, scalar2=1.0,
                                op0=ALU.mult, op1=ALU.add)
        minv = singles.tile([P, T], U8)
        nc.vector.tensor_copy(out=minv, in_=minvf)

        bp = singles.tile([P, T - 1, C], U8)  # slot k <-> step t=k+1

        s = singles.tile([P, C], F32)
        nc.sync.dma_start(out=s, in_=pot[:, 0, :])

        # --- forward ---
        pot_sb = None
        for t in range(1, T):
            if t % CHUNK == 0 or pot_sb is None:
                c0 = (t // CHUNK) * CHUNK
                pot_sb = potp.tile([P, CHUNK, C], F32, tag="pot")
                nc.sync.dma_start(out=pot_sb, in_=pot[:, c0:c0 + CHUNK, :])
            col = t % CHUNK

            x = xp.tile([P, C, C], F32, tag="x")
            nc.vector.tensor_tensor(
                x, at_sb, s.unsqueeze(1).broadcast_to([P, C, C]), ALU.add)
            raw = smal.tile([P, C], F32, tag="raw")
            nc.vector.tensor_reduce(out=raw, in_=x, axis=mybir.AxisListType.X,
                                    op=ALU.max)
            eq = eqp.tile([P, C, C], F32, tag="eq")
            nc.vector.tensor_tensor(
                eq, x, raw.unsqueeze(2).broadcast_to([P, C, C]), ALU.is_equal)
            v = vp.tile([P, C, C], F32, tag="v")
            nc.vector.tensor_tensor(
                v, eq, invcp.unsqueeze(1).broadcast_to([P, C, C]), ALU.mult)
            bpv = smal.tile([P, C], F32, tag="bpv")
            nc.vector.tensor_reduce(out=bpv, in_=v, axis=mybir.AxisListType.X,
                                    op=ALU.max)
            nc.vector.tensor_copy(out=bp[:, t - 1, :], in_=bpv)
            nc.vector.copy_predicated(
                out=bp[:, t - 1, :],
                mask=minv[:, t:t + 1].broadcast_to([P, C]), data=inv_u8)
            tmp = smal.tile([P, C], F32, tag="tmp")
            nc.vector.tensor_tensor(tmp, raw, pot_sb[:, col, :], ALU.add)
            nc.vector.copy_predicated(
                out=s, mask=m[:, t:t + 1].broadcast_to([P, C]), data=tmp)

        # --- last tag (argmax of frozen scores, first-index) ---
        v8 = smal.tile([P, 8], F32, tag="v8")
        nc.vector.max(out=v8, in_=s)
        i8 = smal.tile([P, 8], U32, tag="i8")
        nc.vector.max_index(out=i8, in_max=v8, in_values=s)
        cur0 = singles.tile([P, 1], F32)
        nc.vector.tensor_copy(out=cur0, in_=i8[:, 0:1])
        cur = singles.tile([P, 1], F32)  # 48 - last_tag
        nc.vector.tensor_scalar(out=cur, in0=cur0, scalar1=-1.0,
                                scalar2=float(C), op0=ALU.mult, op1=ALU.add)

        # --- backtrace; one-hot columns written into output chunks ---
        out_sb = outp.tile([P, CHUNK, C], F32, tag="oc")
        nc.vector.tensor_scalar(out=out_sb[:, CHUNK - 1, :], in0=invcp,
                                scalar1=cur[:, :], scalar2=None,
                                op0=ALU.is_equal)
        oh_prev = out_sb[:, CHUNK - 1, :]
        for t in range(T - 2, -1, -1):
            ci = t // CHUNK
            col = t % CHUNK
            if col == CHUNK - 1:
                new_sb = outp.tile([P, CHUNK, C], F32, tag="oc")
                out_sb_hi, out_sb = out_sb, new_sb
            scr = smal.tile([P, C], F32, tag="scr")
            idx = smal.tile([P, 1], F32, tag="idx")
            nc.vector.scalar_tensor_tensor(
                out=scr, in0=bp[:, t, :], scalar=1.0, in1=oh_prev,
                op0=ALU.mult, op1=ALU.mult, accum_out=idx)
            nc.vector.tensor_scalar(out=out_sb[:, col, :], in0=invcp,
                                    scalar1=idx[:, :], scalar2=None,
                                    op0=ALU.is_equal)
            oh_prev = out_sb[:, col, :]
            if col == CHUNK - 1:
                hi0 = (ci + 1) * CHUNK
                nc.sync.dma_start(out=outd[:, hi0:hi0 + CHUNK, :],
                                  in_=out_sb_hi)
        nc.sync.dma_start(out=outd[:, 0:CHUNK, :], in_=out_sb)

    nc.compile()
    return nc


def _get_module():
    if "nc" not in _CACHE:
        _CACHE["nc"] = _build_module()
    return _CACHE["nc"]


def _run(inputs, **spmd_kwargs):
    from concourse.bass_utils import run_bass_kernel_spmd

    potentials = np.ascontiguousarray(inputs["potentials"], dtype=np.float32)
    seq_lens = np.asarray(inputs["sequence_lengths"])
    transitions = np.ascontiguousarray(inputs["transitions"], dtype=np.float32)
    assert potentials.shape == (B, T, C)

    at_host = np.broadcast_to(
        np.ascontiguousarray(transitions.T)[None], (P, C, C))
    at_host = np.ascontiguousarray(at_host)
    lens_f = seq_lens.reshape(B, 1).astype(np.float32)

    in_maps = []
    for c in range(NCORES):
        sl = slice(c * P, (c + 1) * P)
        in_maps.append({
            "pot": np.ascontiguousarray(potentials[sl]),
            "lens": np.ascontiguousarray(lens_f[sl]),
            "at": at_host,
        })

    nc = _get_module()
    res = run_bass_kernel_spmd(nc, in_maps, core_ids=list(range(NCORES)),
                               **spmd_kwargs)
    out = np.concatenate([r["out"] for r in res.results], axis=0)
    return out.astype(np.float32), res


def kernel(**inputs) -> np.ndarray:
    out, _ = _run(inputs)
    return out


# revision 3
# speedup vs baseline: 1.5110x; 1.5110x over previous
"""CRF Viterbi decode (tf.contrib.crf.crf_decode + one_hot) on 8 TRN2 cores.

Data-parallel over batch: each of the 8 NeuronCores processes 128 of the
1024 sequences (batch rows on SBUF partitions). Per core the whole DP is
SBUF-resident and no per-step backpointers are materialized:

  forward (t = 1..511) keeps the full score history shist[:, t, :]:
    x[b,cc,cp] = A_T[cc,cp] + s_{t-1}[b,cp]     tensor_tensor add (s broadcast)
    raw[b,cc]  = max_cp x                       tensor_reduce axis=X
    s_t        = raw + pot[:,t]                 tensor_tensor add
    s_t        = s_{t-1} where t >= len[b]      copy_predicated

  backtrace recomputes only the traced argmax per step:
    ohT  = onehot(tag)^T                        PE transpose
    asel = A[:, tag_b] per batch row            PE matmul (one-hot matvec)
    xcol = s_t + asel                           tensor_tensor (PSUM src)
    tag  = first-argmax(xcol) if t+1 < len      Max8 + MaxIndex + copy_predicated
    out[:, t, :] = onehot(tag)                  tensor_scalar is_equal

Score arithmetic replicates the reference's fp32 ops exactly (same adds,
exact max, first-index argmax), so the output matches bit-for-bit.
"""
import numpy as np

B, T, C = 1024, 512, 48
NCORES = 8
P = B // NCORES  # 128 batch rows per core
CHUNK = 64

_CACHE = {}


def _build_module():
    from contextlib import ExitStack

    import concourse.bacc as bacc
    import concourse.tile as tile
    from concourse import mybir
    from concourse.masks import make_identity

    F32 = mybir.dt.float32
    U8 = mybir.dt.uint8
    I32 = mybir.dt.int32
    U32 = mybir.dt.uint32
    ALU = mybir.AluOpType

    nc = bacc.Bacc("TRN2", debug=False, enable_asserts=False,
                   target_bir_lowering=False, num_devices=NCORES)
    pot = nc.dram_tensor("pot", [P, T, C], F32, kind="ExternalInput").ap()
    lens = nc.dram_tensor("lens", [P, 1], F32, kind="ExternalInput").ap()
    at = nc.dram_tensor("at", [P, C, C], F32, kind="ExternalInput").ap()
    outd = nc.dram_tensor("out", [P, T, C], F32, kind="ExternalOutput").ap()

    with tile.TileContext(nc) as tc, ExitStack() as ctx:
        singles = ctx.enter_context(tc.tile_pool(name="singles", bufs=1))
        potp = ctx.enter_context(tc.tile_pool(name="potp", bufs=2))
        xp = ctx.enter_context(tc.tile_pool(name="xp", bufs=2))
        smal = ctx.enter_context(tc.tile_pool(name="smal", bufs=4))
        outp = ctx.enter_context(tc.tile_pool(name="outp", bufs=2))
        psp = ctx.enter_context(tc.tile_pool(name="psp", bufs=2, space="PSUM"))

        at_sb = singles.tile([P, C, C], F32)
        nc.sync.dma_start(out=at_sb, in_=at)
        at_pe = singles.tile([C, C], F32)  # A_T rows on partitions (PE rhs)
        nc.sync.dma_start(out=at_pe, in_=at[0, :, :])
        lens_sb = singles.tile([P, 1], F32)
        nc.sync.dma_start(out=lens_sb, in_=lens)
        ident = singles.tile([P, P], F32)
        make_identity(nc, ident[:, :])

        iota_t = singles.tile([P, T], I32)
        nc.gpsimd.iota(iota_t, pattern=[[1, T]], base=0, channel_multiplier=0)
        iota_c = singles.tile([P, C], I32)
        nc.gpsimd.iota(iota_c, pattern=[[1, C]], base=0, channel_multiplier=0)
        iota_c_f = singles.tile([P, C], F32)
        nc.vector.tensor_copy(out=iota_c_f, in_=iota_c)

        # m[b,t] = t < len[b]; minv = !m. Integer masks for CopyPredicated
        # (and only plain tensor_copy may write u8 on this HW).
        mf = singles.tile([P, T], F32)
        nc.vector.tensor_scalar(out=mf, in0=iota_t, scalar1=lens_sb[:, :],
                                scalar2=None, op0=ALU.is_lt)
        minvf = singles.tile([P, T], F32)
        nc.vector.tensor_scalar(out=minvf, in0=mf, scalar1=-1.0, scalar2=1.0,
                                op0=ALU.mult, op1=ALU.add)
        m = singles.tile([P, T], U8)
        nc.vector.tensor_copy(out=m, in_=mf)
        minv = singles.tile([P, T], U8)
        nc.vector.tensor_copy(out=minv, in_=minvf)

        # score history: shist[:, t, :] = s_t
        shist = singles.tile([P, T, C], F32)
        nc.sync.dma_start(out=shist[:, 0, :], in_=pot[:, 0, :])

        # --- forward ---
        pot_sb = None
        for t in range(1, T):
            if t % CHUNK == 0 or pot_sb is None:
                c0 = (t // CHUNK) * CHUNK
                pot_sb = potp.tile([P, CHUNK, C], F32, tag="pot")
                nc.sync.dma_start(out=pot_sb, in_=pot[:, c0:c0 + CHUNK, :])
            col = t % CHUNK

            x = xp.tile([P, C, C], F32, tag="x")
            nc.vector.tensor_tensor(
                x, at_sb,
                shist[:, t - 1, :].unsqueeze(1).broadcast_to([P, C, C]),
                ALU.add)
            raw = smal.tile([P, C], F32, tag="raw")
            nc.vector.tensor_reduce(out=raw, in_=x, axis=mybir.AxisListType.X,
                                    op=ALU.max)
            nc.vector.tensor_tensor(shist[:, t, :], raw, pot_sb[:, col, :],
                                    ALU.add)
            nc.vector.copy_predicated(
                out=shist[:, t, :],
                mask=minv[:, t:t + 1].broadcast_to([P, C]),
                data=shist[:, t - 1, :])

        # --- last tag ---
        v8 = smal.tile([P, 8], F32, tag="v8")
        nc.vector.max(out=v8, in_=shist[:, T - 1, :])
        i8 = smal.tile([P, 8], U32, tag="i8")
        nc.vector.max_index(out=i8, in_max=v8, in_values=shist[:, T - 1, :])
        tag = singles.tile([P, 1], F32)
        nc.vector.tensor_copy(out=tag, in_=i8[:, 0:1])

        # --- backtrace ---
        out_sb = outp.tile([P, CHUNK, C], F32, tag="oc")
        nc.vector.tensor_scalar(out=out_sb[:, CHUNK - 1, :], in0=iota_c_f,
                                scalar1=tag[:, :], scalar2=None,
                                op0=ALU.is_equal)
        oh_prev = out_sb[:, CHUNK - 1, :]
        for t in range(T - 2, -1, -1):
            ci = t // CHUNK
            col = t % CHUNK
            if col == CHUNK - 1:
                new_sb = outp.tile([P, CHUNK, C], F32, tag="oc")
                out_sb_hi, out_sb = out_sb, new_sb
            ps_ohT = psp.tile([C, P], F32, tag="ohT")
            nc.tensor.transpose(ps_ohT, oh_prev, ident[:, :])
            sb_ohT = smal.tile([C, P], F32, tag="sbohT")
            nc.scalar.copy(out=sb_ohT, in_=ps_ohT)
            ps_asel = psp.tile([P, C], F32, tag="asel")
            nc.tensor.matmul(ps_asel, sb_ohT, at_pe, start=True, stop=True)
            xcol = smal.tile([P, C], F32, tag="xcol")
            nc.vector.tensor_tensor(xcol, shist[:, t, :], ps_asel, ALU.add)
            bv8 = smal.tile([P, 8], F32, tag="bv8")
            nc.vector.max(out=bv8, in_=xcol)
            bi8 = smal.tile([P, 8], U32, tag="bi8")
            nc.vector.max_index(out=bi8, in_max=bv8, in_values=xcol)
            tag_new = smal.tile([P, 1], F32, tag="tagn")
            nc.vector.tensor_copy(out=tag_new, in_=bi8[:, 0:1])
            nc.vector.copy_predicated(out=tag, mask=m[:, t + 1:t + 2],
                                      data=tag_new)
            nc.vector.tensor_scalar(out=out_sb[:, col, :], in0=iota_c_f,
                                    scalar1=tag[:, :], scalar2=None,
                                    op0=ALU.is_equal)
            oh_prev = out_sb[:, col, :]
            if col == CHUNK - 1:
                hi0 = (ci + 1) * CHUNK
                nc.sync.dma_start(out=outd[:, hi0:hi0 + CHUNK, :],
                                  in_=out_sb_hi)
        nc.sync.dma_start(out=outd[:, 0:CHUNK, :], in_=out_sb)

    nc.compile()
    return nc


def _get_module():
    if "nc" not in _CACHE:
        _CACHE["nc"] = _build_module()
    return _CACHE["nc"]


def _run(inputs, **spmd_kwargs):
    from concourse.bass_utils import run_bass_kernel_spmd

    potentials = np.ascontiguousarray(inputs["potentials"], dtype=np.float32)
    seq_lens = np.asarray(inputs["sequence_lengths"])
    transitions = np.ascontiguousarray(inputs["transitions"], dtype=np.float32)
    assert potentials.shape == (B, T, C)

    at_host = np.broadcast_to(
        np.ascontiguousarray(transitions.T)[None], (P, C, C))
    at_host = np.ascontiguousarray(at_host)
    lens_f = seq_lens.reshape(B, 1).astype(np.float32)

    in_maps = []
    for c in range(NCORES):
        sl = slice(c * P, (c + 1) * P)
        in_maps.append({
            "pot": np.ascontiguousarray(potentials[sl]),
            "lens": np.ascontiguousarray(lens_f[sl]),
            "at": at_host,
        })

    nc = _get_module()
    res = run_bass_kernel_spmd(nc, in_maps, core_ids=list(range(NCORES)),
                               **spmd_kwargs)
    out = np.concatenate([r["out"] for r in res.results], axis=0)
    return out.astype(np.float32), res


def kernel(**inputs) -> np.ndarray:
    out, _ = _run(inputs)
    return out


# revision 5
# speedup vs baseline: 1.5132x; 1.0014x over previous
"""CRF Viterbi decode (tf.contrib.crf.crf_decode + one_hot) on 8 TRN2 cores.

Data-parallel over batch: each of the 8 NeuronCores processes 128 of the
1024 sequences (batch rows on SBUF partitions). Per core the whole DP is
SBUF-resident and no per-step backpointers are materialized:

  forward (t = 1..511) keeps the full score history shist[:, t, :]:
    x[b,cc,cp] = A_T[cc,cp] + s_{t-1}[b,cp]     tensor_tensor add (s broadcast)
    raw[b,cc]  = max_cp x                       tensor_reduce axis=X
    s_t        = raw + pot[:,t]                 tensor_tensor add
    s_t        = s_{t-1} where t >= len[b]      copy_predicated

  backtrace recomputes only the traced argmax per step:
    ohT  = onehot(tag)^T (bf16)                 PE transpose
    asel = A[:, tag_b] per batch row            PE matmul (one-hot matvec, f32)
    xcol = s_t + asel                           tensor_tensor (PSUM src)
    tag  = first-argmax(xcol) if t+1 < len      Max8 + MaxIndex + copy_predicated
    out[:, t, :] = onehot(tag)                  tensor_scalar is_equal (+cast)

Score arithmetic replicates the reference's fp32 ops exactly (same adds,
exact max via one-hot matvec with exact 0/1 weights, first-index argmax),
so the output matches bit-for-bit.
"""
from contextlib import ExitStack

import numpy as np

B, T, C = 1024, 512, 48
NCORES = 8
P = B // NCORES  # 128 batch rows per core
CHUNK = 64

_CACHE = {}


def crf_body(tc, outs, ins, T=T, CHUNK=CHUNK):
    import concourse.tile as tile  # noqa: F401
    from concourse import mybir
    from concourse.masks import make_identity

    F32 = mybir.dt.float32
    BF16 = mybir.dt.bfloat16
    U8 = mybir.dt.uint8
    I32 = mybir.dt.int32
    U32 = mybir.dt.uint32
    ALU = mybir.AluOpType

    nc = tc.nc
    pot = ins["pot"]      # [P, T, C] f32 dram
    lens = ins["lens"]    # [P, 1] f32 dram
    at = ins["at"]        # [P, C, C] f32 dram  (at[b,cc,cp] = A[cp,cc])
    outd = outs["out"]    # [P, T, C] f32 dram
    assert T % CHUNK == 0

    with ExitStack() as ctx:
        singles = ctx.enter_context(tc.tile_pool(name="singles", bufs=1))
        potp = ctx.enter_context(tc.tile_pool(name="potp", bufs=2))
        xp = ctx.enter_context(tc.tile_pool(name="xp", bufs=2))
        smal = ctx.enter_context(tc.tile_pool(name="smal", bufs=4))
        outp = ctx.enter_context(tc.tile_pool(name="outp", bufs=2))
        psp = ctx.enter_context(tc.tile_pool(name="psp", bufs=2, space="PSUM"))

        at_sb = singles.tile([P, C, C], F32)
        nc.sync.dma_start(out=at_sb, in_=at)
        at_pe = singles.tile([C, C], F32)  # A_T rows on partitions (PE rhs)
        nc.sync.dma_start(out=at_pe, in_=at[0, :, :])
        lens_sb = singles.tile([P, 1], F32)
        nc.sync.dma_start(out=lens_sb, in_=lens)
        ident = singles.tile([P, P], BF16)
        make_identity(nc, ident[:, :])

        iota_t = singles.tile([P, T], I32)
        nc.gpsimd.iota(iota_t, pattern=[[1, T]], base=0, channel_multiplier=0)
        iota_c = singles.tile([P, C], I32)
        nc.gpsimd.iota(iota_c, pattern=[[1, C]], base=0, channel_multiplier=0)
        iota_c_f = singles.tile([P, C], F32)
        nc.vector.tensor_copy(out=iota_c_f, in_=iota_c)

        # m[b,t] = t < len[b]; minv = !m. Integer masks for CopyPredicated
        # (and only plain tensor_copy may write u8 on this HW).
        mf = singles.tile([P, T], F32)
        nc.vector.tensor_scalar(out=mf, in0=iota_t, scalar1=lens_sb[:, :],
                                scalar2=None, op0=ALU.is_lt)
        minvf = singles.tile([P, T], F32)
        nc.vector.tensor_scalar(out=minvf, in0=mf, scalar1=-1.0, scalar2=1.0,
                                op0=ALU.mult, op1=ALU.add)
        m = singles.tile([P, T], U8)
        nc.vector.tensor_copy(out=m, in_=mf)
        minv = singles.tile([P, T], U8)
        nc.vector.tensor_copy(out=minv, in_=minvf)

        # score history: shist[:, t, :] = s_t
        shist = singles.tile([P, T, C], F32)
        nc.sync.dma_start(out=shist[:, 0, :], in_=pot[:, 0, :])

        # --- forward ---
        pot_sb = None
        for t in range(1, T):
            if t % CHUNK == 0 or pot_sb is None:
                c0 = (t // CHUNK) * CHUNK
                pot_sb = potp.tile([P, CHUNK, C], F32, tag="pot")
                nc.sync.dma_start(out=pot_sb, in_=pot[:, c0:c0 + CHUNK, :])
            col = t % CHUNK

            x = xp.tile([P, C, C], F32, tag="x")
            nc.vector.tensor_tensor(
                x, at_sb,
                shist[:, t - 1, :].unsqueeze(1).broadcast_to([P, C, C]),
                ALU.add)
            raw = smal.tile([P, C], F32, tag="raw")
            nc.vector.tensor_reduce(out=raw, in_=x, axis=mybir.AxisListType.X,
                                    op=ALU.max)
            nc.vector.tensor_tensor(shist[:, t, :], raw, pot_sb[:, col, :],
                                    ALU.add)
            nc.vector.copy_predicated(
                out=shist[:, t, :],
                mask=minv[:, t:t + 1].broadcast_to([P, C]),
                data=shist[:, t - 1, :])

        # --- last tag ---
        v8 = smal.tile([P, 8], F32, tag="v8")
        nc.vector.max(out=v8, in_=shist[:, T - 1, :])
        i8 = smal.tile([P, 8], U32, tag="i8")
        nc.vector.max_index(out=i8, in_max=v8, in_values=shist[:, T - 1, :])
        tag = singles.tile([P, 1], F32)
        nc.vector.tensor_copy(out=tag, in_=i8[:, 0:1])

        # --- backtrace ---
        # bf16 one-hot state: exact for 0/1, and the PE transpose of bf16 is
        # a single matmul pass (fp32 needs two).
        oh = smal.tile([P, C], BF16, tag="oh")
        nc.vector.tensor_scalar(out=oh, in0=iota_c_f, scalar1=tag[:, :],
                                scalar2=None, op0=ALU.is_equal)
        out_sb = outp.tile([P, CHUNK, C], F32, tag="oc")
        nc.scalar.copy(out=out_sb[:, CHUNK - 1, :], in_=oh)
        for t in range(T - 2, -1, -1):
            ci = t // CHUNK
            col = t % CHUNK
            if col == CHUNK - 1:
                new_sb = outp.tile([P, CHUNK, C], F32, tag="oc")
                out_sb_hi, out_sb = out_sb, new_sb
            ps_ohT = psp.tile([C, P], BF16, tag="ohT")
            nc.tensor.transpose(ps_ohT, oh, ident[:, :])
            sb_ohT = smal.tile([C, P], F32, tag="sbohT")
            nc.scalar.copy(out=sb_ohT, in_=ps_ohT)
            ps_asel = psp.tile([P, C], F32, tag="asel")
            nc.tensor.matmul(ps_asel, sb_ohT, at_pe, start=True, stop=True)
            xcol = smal.tile([P, C], F32, tag="xcol")
            nc.vector.tensor_tensor(xcol, shist[:, t, :], ps_asel, ALU.add)
            bv8 = smal.tile([P, 8], F32, tag="bv8")
            nc.vector.max(out=bv8, in_=xcol)
            bi8 = smal.tile([P, 8], U32, tag="bi8")
            nc.vector.max_index(out=bi8, in_max=bv8, in_values=xcol)
            tag_new = smal.tile([P, 1], F32, tag="tagn")
            nc.vector.tensor_copy(out=tag_new, in_=bi8[:, 0:1])
            nc.vector.copy_predicated(out=tag, mask=m[:, t + 1:t + 2],
                                      data=tag_new)
            oh = smal.tile([P, C], BF16, tag="oh")
            nc.vector.tensor_scalar(out=oh, in0=iota_c_f, scalar1=tag[:, :],
                                    scalar2=None, op0=ALU.is_equal)
            # output column (f32) off the critical chain, on the Scalar engine
            nc.scalar.copy(out=out_sb[:, col, :], in_=oh)
            if col == CHUNK - 1:
                hi0 = (ci + 1) * CHUNK
                nc.sync.dma_start(out=outd[:, hi0:hi0 + CHUNK, :],
                                  in_=out_sb_hi)
        nc.sync.dma_start(out=outd[:, 0:CHUNK, :], in_=out_sb)


def _build_module():
    import concourse.bacc as bacc
    import concourse.tile as tile
    from concourse import mybir

    F32 = mybir.dt.float32
    nc = bacc.Bacc("TRN2", debug=False, enable_asserts=False,
                   target_bir_lowering=False, num_devices=NCORES)
    ins = {
        "pot": nc.dram_tensor("pot", [P, T, C], F32, kind="ExternalInput").ap(),
        "lens": nc.dram_tensor("lens", [P, 1], F32, kind="ExternalInput").ap(),
        "at": nc.dram_tensor("at", [P, C, C], F32, kind="ExternalInput").ap(),
    }
    outs = {
        "out": nc.dram_tensor("out", [P, T, C], F32, kind="ExternalOutput").ap(),
    }
    with tile.TileContext(nc) as tc:
        crf_body(tc, outs, ins)
    nc.compile()
    return nc


def _get_module():
    if "nc" not in _CACHE:
        _CACHE["nc"] = _build_module()
    return _CACHE["nc"]


def _run(inputs, **spmd_kwargs):
    from concourse.bass_utils import run_bass_kernel_spmd

    potentials = np.ascontiguousarray(inputs["potentials"], dtype=np.float32)
    seq_lens = np.asarray(inputs["sequence_lengths"])
    transitions = np.ascontiguousarray(inputs["transitions"], dtype=np.float32)
    assert potentials.shape == (B, T, C)

    at_host = np.broadcast_to(
        np.ascontiguousarray(transitions.T)[None], (P, C, C))
    at_host = np.ascontiguousarray(at_host)
    lens_f = seq_lens.reshape(B, 1).astype(np.float32)

    in_maps = []
    for c in range(NCORES):
        sl = slice(c * P, (c + 1) * P)
        in_maps.append({
            "pot": np.ascontiguousarray(potentials[sl]),
            "lens": np.ascontiguousarray(lens_f[sl]),
            "at": at_host,
        })

    nc = _get_module()
    res = run_bass_kernel_spmd(nc, in_maps, core_ids=list(range(NCORES)),
                               **spmd_kwargs)
    out = np.concatenate([r["out"] for r in res.results], axis=0)
    return out.astype(np.float32), res


def kernel(**inputs) -> np.ndarray:
    out, _ = _run(inputs)
    return out


# revision 6
# speedup vs baseline: 2.1273x; 1.4058x over previous
"""CRF Viterbi decode (tf.contrib.crf.crf_decode + one_hot) on 8 TRN2 cores.

Data-parallel over batch: each of the 8 NeuronCores processes 128 of the
1024 sequences (batch rows on SBUF partitions). Per core the whole DP is
SBUF-resident and no per-step backpointers are materialized:

  forward (t = 1..511) keeps the full score history shist[:, t, :]:
    x[b,cc,cp] = A_T[cc,cp] + s_{t-1}[b,cp]     tensor_tensor add (s broadcast)
    raw[b,cc]  = max_cp x                       tensor_reduce axis=X
    s_t        = raw + pot[:,t]                 tensor_tensor add
    s_t        = s_{t-1} where t >= len[b]      copy_predicated

  backtrace recomputes only the traced argmax per step:
    ohT  = onehot(tag)^T (bf16)                 PE transpose
    asel = A[:, tag_b] per batch row            PE matmul (one-hot matvec, f32)
    xcol = s_t + asel                           tensor_tensor (PSUM src)
    tag  = first-argmax(xcol) if t+1 < len      Max8 + MaxIndex + copy_predicated
    out[:, t, :] = onehot(tag)                  tensor_scalar is_equal (+cast)

Score arithmetic replicates the reference's fp32 ops exactly (same adds,
exact max via one-hot matvec with exact 0/1 weights, first-index argmax),
so the output matches bit-for-bit.
"""
from contextlib import ExitStack

import numpy as np

B, T, C = 1024, 512, 48
NCORES = 8
P = B // NCORES  # 128 batch rows per core
CHUNK = 64

_CACHE = {}


def _register_seg_maxplus():
    """Custom DVE op: one streaming pass computing, per partition, the
    SEGMENTED running max of (in0 + in1) with segment length 48 (SUB_DIM
    boundaries of the 3D in0 AP). The segment tails are the grouped maxes.

    The stock Spec DSL has no segmented-scan reset, but the generated FSM
    already supports a SUB_DIM_DONE-triggered step state with per-stage
    overrides (production-tested by TENSOR_PAGED_MASK). We patch
    `_scan_overrides` to emit, for scans tagged `_ant_reset`, a step
    override that re-seeds the scan register from the current element:
    state := expr  (instead of state := max(state, expr)).
    """
    import concourse.dve_spec as ds
    from concourse.dve_ops import (OPS, CUSTOM_DVE_SPECS, DveOp,
                                   _SUB_OPCODE_FOR_NAME)
    from concourse.dve_uop import DveOpSpec

    name = "SEG_MAXPLUS_ANT"
    for o in OPS:
        if o.name == name:
            return o

    if not getattr(ds, "_ant_seg_patch", False):
        _orig_overrides = ds._scan_overrides

        def _patched(scans, node_stage):
            seed, step = _orig_overrides(scans, node_stage)
            for sc in scans:
                if getattr(sc, "_ant_reset", False):
                    step[node_stage[sc]] = ds._Stage(ds.AluOp.BYPASS, sc.expr)
            return seed, step

        ds._scan_overrides = _patched
        ds._ant_seg_patch = True

    body = ds.scan(ds.AluOp.MAX, ds.Src0 + ds.Src1)
    object.__setattr__(body, "_ant_reset", True)

    def _ref(in0, in1, c0, c1, c2):
        x = np.asarray(in0, np.float32) + np.asarray(in1, np.float32).reshape(
            in0.shape)
        p = x.shape[0]
        x3 = x.reshape(p, -1, x.shape[-1])
        return np.maximum.accumulate(x3, axis=-1).reshape(in0.shape)

    spec = ds.Spec(body=body, reference=_ref)
    row = 1 + len(OPS)
    _SUB_OPCODE_FOR_NAME[name] = row
    shas = {}
    for ver in ("v3", "v4"):
        try:
            shas[ver] = DveOpSpec(name=name, opcode=row,
                                  uops=ds.lower(spec, ver=ver),
                                  rd1_en=True).sha(ver)
        except Exception:
            pass
    op = DveOp(name, spec, subdim=True, uops_sha=shas)
    OPS.append(op)
    CUSTOM_DVE_SPECS[name] = spec
    return op


def crf_body(tc, outs, ins, T=T, CHUNK=CHUNK):
    import concourse.tile as tile  # noqa: F401
    from concourse import mybir
    from concourse.masks import make_identity

    F32 = mybir.dt.float32
    BF16 = mybir.dt.bfloat16
    U8 = mybir.dt.uint8
    I32 = mybir.dt.int32
    U32 = mybir.dt.uint32
    ALU = mybir.AluOpType

    nc = tc.nc
    segop = _register_seg_maxplus()
    pot = ins["pot"]      # [P, T, C] f32 dram
    lens = ins["lens"]    # [P, 1] f32 dram
    at = ins["at"]        # [P, C, C] f32 dram  (at[b,cc,cp] = A[cp,cc])
    outd = outs["out"]    # [P, T, C] f32 dram
    assert T % CHUNK == 0

    with ExitStack() as ctx:
        singles = ctx.enter_context(tc.tile_pool(name="singles", bufs=1))
        potp = ctx.enter_context(tc.tile_pool(name="potp", bufs=2))
        xp = ctx.enter_context(tc.tile_pool(name="xp", bufs=2))
        smal = ctx.enter_context(tc.tile_pool(name="smal", bufs=4))
        outp = ctx.enter_context(tc.tile_pool(name="outp", bufs=2))
        psp = ctx.enter_context(tc.tile_pool(name="psp", bufs=2, space="PSUM"))

        at_sb = singles.tile([P, C, C], F32)
        nc.sync.dma_start(out=at_sb, in_=at)
        at_pe = singles.tile([C, C], F32)  # A_T rows on partitions (PE rhs)
        nc.sync.dma_start(out=at_pe, in_=at[0, :, :])
        lens_sb = singles.tile([P, 1], F32)
        nc.sync.dma_start(out=lens_sb, in_=lens)
        ident = singles.tile([P, P], BF16)
        make_identity(nc, ident[:, :])

        iota_t = singles.tile([P, T], I32)
        nc.gpsimd.iota(iota_t, pattern=[[1, T]], base=0, channel_multiplier=0)
        iota_c = singles.tile([P, C], I32)
        nc.gpsimd.iota(iota_c, pattern=[[1, C]], base=0, channel_multiplier=0)
        iota_c_f = singles.tile([P, C], F32)
        nc.vector.tensor_copy(out=iota_c_f, in_=iota_c)

        # m[b,t] = t < len[b]; minv = !m. Integer masks for CopyPredicated
        # (and only plain tensor_copy may write u8 on this HW).
        mf = singles.tile([P, T], F32)
        nc.vector.tensor_scalar(out=mf, in0=iota_t, scalar1=lens_sb[:, :],
                                scalar2=None, op0=ALU.is_lt)
        minvf = singles.tile([P, T], F32)
        nc.vector.tensor_scalar(out=minvf, in0=mf, scalar1=-1.0, scalar2=1.0,
                                op0=ALU.mult, op1=ALU.add)
        m = singles.tile([P, T], U8)
        nc.vector.tensor_copy(out=m, in_=mf)
        minv = singles.tile([P, T], U8)
        nc.vector.tensor_copy(out=minv, in_=minvf)

        # score history: shist[:, t, :] = s_t
        shist = singles.tile([P, T, C], F32)
        nc.sync.dma_start(out=shist[:, 0, :], in_=pot[:, 0, :])

        # --- forward ---
        pot_sb = None
        for t in range(1, T):
            if t % CHUNK == 0 or pot_sb is None:
                c0 = (t // CHUNK) * CHUNK
                pot_sb = potp.tile([P, CHUNK, C], F32, tag="pot")
                nc.sync.dma_start(out=pot_sb, in_=pot[:, c0:c0 + CHUNK, :])
            col = t % CHUNK

            x = xp.tile([P, C, C], F32, tag="x")
            nc.vector._custom_dve(
                segop, out=x, in0=at_sb[:, :, :],
                in1=shist[:, t - 1, :].unsqueeze(1).broadcast_to([P, C, C]))
            nc.vector.tensor_tensor(shist[:, t, :], x[:, :, C - 1],
                                    pot_sb[:, col, :], ALU.add)
            nc.vector.copy_predicated(
                out=shist[:, t, :],
                mask=minv[:, t:t + 1].broadcast_to([P, C]),
                data=shist[:, t - 1, :])

        # --- last tag ---
        v8 = smal.tile([P, 8], F32, tag="v8")
        nc.vector.max(out=v8, in_=shist[:, T - 1, :])
        i8 = smal.tile([P, 8], U32, tag="i8")
        nc.vector.max_index(out=i8, in_max=v8, in_values=shist[:, T - 1, :])
        tag = singles.tile([P, 1], F32)
        nc.vector.tensor_copy(out=tag, in_=i8[:, 0:1])

        # --- backtrace ---
        # bf16 one-hot state: exact for 0/1, and the PE transpose of bf16 is
        # a single matmul pass (fp32 needs two).
        oh = smal.tile([P, C], BF16, tag="oh")
        nc.vector.tensor_scalar(out=oh, in0=iota_c_f, scalar1=tag[:, :],
                                scalar2=None, op0=ALU.is_equal)
        out_sb = outp.tile([P, CHUNK, C], F32, tag="oc")
        nc.scalar.copy(out=out_sb[:, CHUNK - 1, :], in_=oh)
        for t in range(T - 2, -1, -1):
            ci = t // CHUNK
            col = t % CHUNK
            if col == CHUNK - 1:
                new_sb = outp.tile([P, CHUNK, C], F32, tag="oc")
                out_sb_hi, out_sb = out_sb, new_sb
            ps_ohT = psp.tile([C, P], BF16, tag="ohT")
            nc.tensor.transpose(ps_ohT, oh, ident[:, :])
            sb_ohT = smal.tile([C, P], F32, tag="sbohT")
            nc.scalar.copy(out=sb_ohT, in_=ps_ohT)
            ps_asel = psp.tile([P, C], F32, tag="asel")
            nc.tensor.matmul(ps_asel, sb_ohT, at_pe, start=True, stop=True)
            xcol = smal.tile([P, C], F32, tag="xcol")
            nc.vector.tensor_tensor(xcol, shist[:, t, :], ps_asel, ALU.add)
            bv8 = smal.tile([P, 8], F32, tag="bv8")
            nc.vector.max(out=bv8, in_=xcol)
            bi8 = smal.tile([P, 8], U32, tag="bi8")
            nc.vector.max_index(out=bi8, in_max=bv8, in_values=xcol)
            tag_new = smal.tile([P, 1], F32, tag="tagn")
            nc.vector.tensor_copy(out=tag_new, in_=bi8[:, 0:1])
            nc.vector.copy_predicated(out=tag, mask=m[:, t + 1:t + 2],
                                      data=tag_new)
            oh = smal.tile([P, C], BF16, tag="oh")
            nc.vector.tensor_scalar(out=oh, in0=iota_c_f, scalar1=tag[:, :],
                                    scalar2=None, op0=ALU.is_equal)
            # output column (f32) off the critical chain, on the Scalar engine
            nc.scalar.copy(out=out_sb[:, col, :], in_=oh)
            if col == CHUNK - 1:
                hi0 = (ci + 1) * CHUNK
                nc.sync.dma_start(out=outd[:, hi0:hi0 + CHUNK, :],
                                  in_=out_sb_hi)
        nc.sync.dma_start(out=outd[:, 0:CHUNK, :], in_=out_sb)


def _build_module():
    import concourse.bacc as bacc
    import concourse.tile as tile
    from concourse import mybir

    F32 = mybir.dt.float32
    nc = bacc.Bacc("TRN2", debug=False, enable_asserts=False,
                   target_bir_lowering=False, num_devices=NCORES)
    ins = {
        "pot": nc.dram_tensor("pot", [P, T, C], F32, kind="ExternalInput").ap(),
        "lens": nc.dram_tensor("lens", [P, 1], F32, kind="ExternalInput").ap(),
        "at": nc.dram_tensor("at", [P, C, C], F32, kind="ExternalInput").ap(),
    }
    outs = {
        "out": nc.dram_tensor("out", [P, T, C], F32, kind="ExternalOutput").ap(),
    }
    with tile.TileContext(nc) as tc:
        crf_body(tc, outs, ins)
    nc.compile()
    return nc


def _get_module():
    if "nc" not in _CACHE:
        _CACHE["nc"] = _build_module()
    return _CACHE["nc"]


def _run(inputs, **spmd_kwargs):
    from concourse.bass_utils import run_bass_kernel_spmd

    potentials = np.ascontiguousarray(inputs["potentials"], dtype=np.float32)
    seq_lens = np.asarray(inputs["sequence_lengths"])
    transitions = np.ascontiguousarray(inputs["transitions"], dtype=np.float32)
    assert potentials.shape == (B, T, C)

    at_host = np.broadcast_to(
        np.ascontiguousarray(transitions.T)[None], (P, C, C))
    at_host = np.ascontiguousarray(at_host)
    lens_f = seq_lens.reshape(B, 1).astype(np.float32)

    in_maps = []
    for c in range(NCORES):
        sl = slice(c * P, (c + 1) * P)
        in_maps.append({
            "pot": np.ascontiguousarray(potentials[sl]),
            "lens": np.ascontiguousarray(lens_f[sl]),
            "at": at_host,
        })

    nc = _get_module()
    res = run_bass_kernel_spmd(nc, in_maps, core_ids=list(range(NCORES)),
                               **spmd_kwargs)
    out = np.concatenate([r["out"] for r in res.results], axis=0)
    return out.astype(np.float32), res


def kernel(**inputs) -> np.ndarray:
    out, _ = _run(inputs)
    return out
